# revision 21
# baseline (speedup 1.0000x reference)
"""Multi-head attention forward, distributed over 8 TRN2 NeuronCores.

Problem: x[2,2048,1024] -> QKV proj (16 heads x 64) -> softmax attention
-> output proj + bias -> [2,2048,1024], f32 I/O, bf16 tensor-engine compute.

Sharding: rows = flattened (batch, seq) = 4096 rows; core c owns rows
[c*512, (c+1)*512) -- cores 0-3 hold batch 0, cores 4-7 batch 1. Each core
computes attention for its 512 query rows over all 2048 keys of its batch.

v8 schedule. Measured constraints this is built around:
  - ACT exp stream paces attention: ~1.2us per [128,1024] exp, 128 exps.
  - bf16 matmul sustains ~250ns at N=512 for any K<=128/M<=128.
  - Collectives start ~10us after their trigger (the "barrier" just waits
    for the first trigger), and run serially on the CC stream. Pairwise
    exchange with rank^1 is intra-chip: 4MB gathered in 28us (149GB/s).
    rank^2/rank^3 pairwise crosses chips at only ~55GB/s, but the 4-way
    ring AllGather pipelines the cross links well (1MB in -> 4MB out in
    ~45us). A gather's wire traffic starves concurrent unpack DMAs
    (they creep at ~1/3 speed).
  - Engine queues are in-order; att matmuls lag their pair's scores by
    one pair so data-waits on V never block the next scores.
Structure:
  - One packed block kv_in = [K^T | V] in two key-halves. Three
    collectives fire back-to-back off it: intra-chip pairwise AllGather
    (buddy's full block, lands ~75us), then two 4-way AllGathers (key
    halves; the far pair's data lands ~120/~165us).
  - Attention passes: own keys (from SBUF, exp starts ~45us, Q^T
    projection interleaved pair-by-pair), buddy keys, far pair's first
    key-half, far pair's second key-half. Arrival times match each
    pass's start, so the exp stream runs nearly gap-free.
  - Rank-dependent unpack offsets use dynamic-offset DMAs driven by
    partition_id; v_aug tiles rotate through 8 buffers (a pass's V dies
    when its att matmuls drain).
Layouts as v1 (S^T [keys, q] scores, zero-padded qTe/qTo pair trick for
full-rate K=128 matmuls, V augmented with a ones column so row 64 of
att^T is the softmax denominator; exp folds the 1/sqrt(64) scale, no max
subtraction since scores ~N(0,1)).
"""

import ml_dtypes
import numpy as np

import concourse.bass as bass
import concourse.mybir as mybir
import concourse.tile as tile
from concourse import bacc
from concourse.bass_utils import run_bass_kernel_spmd

BF = mybir.dt.bfloat16
F32 = mybir.dt.float32
P = 128

N_CORES = 8
GROUP = 4   # cores per batch group


class Cfg:
    def __init__(self, rpc, d, n_heads, head_dim):
        self.RPC = rpc            # query rows per core
        self.D = d                # model dim
        self.H = n_heads
        self.HD = head_dim
        assert n_heads * head_dim == d
        self.NT_D = d // P        # dim tiles (= head pairs)
        self.NT_R = rpc // P      # row tiles (= local key tiles)
        self.KEYS = rpc * GROUP   # keys per batch group
        self.NT_K = self.KEYS // P
        assert P // head_dim == 2
        # kv_in: two key-half chunks of [KTW*NT_D K rows + 2*VTW V rows]
        self.KTW = 64             # rows of one K^T m-part per chunk
        self.VTW = 256            # rows of one packed V row-tile
        self.CHR = self.NT_D * self.KTW + 2 * self.VTW  # 1024
        self.BLK = 2 * self.CHR   # 2048 rows per rank block


FULL = Cfg(rpc=512, d=1024, n_heads=16, head_dim=64)


def _body(tc, nc, cfg, xT_in, wq_in, wk_in, wv_in, wo_in, bo_in, out_ext):
    c = cfg
    AF = mybir.ActivationFunctionType
    HD1 = c.HD + 1
    from contextlib import ExitStack

    stack = ExitStack()
    dram = stack.enter_context(tc.tile_pool(name="dram", bufs=1, space="DRAM"))
    const = stack.enter_context(tc.tile_pool(name="const", bufs=1))
    persist = stack.enter_context(tc.tile_pool(name="persist", bufs=1))

    kv_in = dram.tile([c.BLK, c.RPC], BF, name="kv_in")
    r1_g = dram.tile([2 * c.BLK, c.RPC], BF, name="r1_g")
    cg = [dram.tile([GROUP * c.CHR, c.RPC], BF, name=f"cg{h}") for h in range(2)]

    ones_row = const.tile([1, P], BF, tag="ones_row", name="ones_row")
    nc.vector.memset(ones_row[:], 1.0)
    bo_sb = const.tile([1, c.D], BF, tag="bo", name="bo_sb")
    nc.sync.dma_start(bo_sb[:], bo_in[:, :])

    def ptiles(shape, dt_, pfx, n, pool=None):
        pool = pool or persist
        return [pool.tile(shape, dt_, tag=f"{pfx}{t}", name=f"{pfx}{t}") for t in range(n)]

    # wide single tiles -> one DMA per input (SP descriptor time matters)
    xT_all = persist.tile([P, c.NT_D * c.RPC], BF, tag="xT", name="xT")
    xT = [xT_all[:, t * c.RPC : (t + 1) * c.RPC] for t in range(c.NT_D)]
    wo_all = persist.tile([P, c.NT_D * c.D], BF, tag="wo", name="wo")
    wo_sb = [wo_all[:, t * c.D : (t + 1) * c.D] for t in range(c.NT_D)]
    wv_all = persist.tile([P, c.NT_D * c.D], BF, tag="wv", name="wv")
    wv_sb = [wv_all[:, t * c.D : (t + 1) * c.D] for t in range(c.NT_D)]
    wq_all = persist.tile([P, c.NT_D * c.D], BF, tag="wq", name="wq")
    wq_sb = [wq_all[:, t * c.D : (t + 1) * c.D] for t in range(c.NT_D)]

    kt_loc = ptiles([P, c.RPC], BF, "ktl", c.NT_D)
    # remote K^T: per pair, slot s (0=buddy, 1/2=far) at cols s*RPC
    kt_rem = ptiles([P, 3 * c.RPC], BF, "ktr", c.NT_D)
    qTe = ptiles([P, c.RPC], BF, "qTe", c.NT_D)
    qTo = ptiles([P, c.RPC], BF, "qTo", c.NT_D)
    vloc = ptiles([P, c.D], BF, "vloc", c.NT_R)
    # v_aug: 8 rotating tiles; remote slot s key-tile j -> (4+s*4+j) % 8
    v_aug_t = ptiles([P, c.H * HD1], BF, "va", 8)
    v_aug = lambda j: v_aug_t[j % 8]
    attT = ptiles([P, c.RPC], BF, "attT", c.NT_D)
    acc = ptiles([HD1, 2 * c.RPC], BF, "acc", c.NT_D)

    def build_v_aug(j, src):
        nc.vector.tensor_copy(
            v_aug(j)[:].rearrange("p (h e) -> p h e", e=HD1)[:, :, 0 : c.HD],
            src.rearrange("p (h e) -> p h e", e=c.HD),
        )
        ones_col = v_aug(j)[:].rearrange("p (h e) -> p h e", e=HD1)[:, :, c.HD : HD1]
        nc.vector.memset(ones_col, 1.0)

    def load_wide(dst_all, src_ap):
        w = src_ap.shape[1]
        nc.sync.dma_start(
            dst_all[:].rearrange("p (t w) -> p t w", w=w),
            src_ap[:, :].rearrange("(t p) w -> p t w", p=P),
        )

    load_wide(xT_all, xT_in)

    with (
        tc.tile_pool(name="wk_pool", bufs=1) as wk_pool,
        tc.tile_pool(name="proj_psum", bufs=4, space="PSUM") as proj_psum,
        tc.tile_pool(name="projv_psum", bufs=2, space="PSUM") as projv_psum,
    ):
        wk_all = wk_pool.tile([P, c.NT_D * c.D], BF, tag="wk", name="wk")
        wk_sb = [wk_all[:, t * c.D : (t + 1) * c.D] for t in range(c.NT_D)]
        load_wide(wk_all, wk_in)
        load_wide(wv_all, wv_in)
        load_wide(wq_all, wq_in)
        load_wide(wo_all, wo_in)

        # ---- K then V projections; pack kv_in as we go ----
        for m in range(c.NT_D):
            ps = proj_psum.tile([P, c.RPC], F32, tag="proj", name="proj_ps")
            for k in range(c.NT_D):
                nc.tensor.matmul(
                    ps[:],
                    wk_sb[k][:, m * P : (m + 1) * P],
                    xT[k][:],
                    start=(k == 0),
                    stop=(k == c.NT_D - 1),
                )
            nc.vector.tensor_copy(kt_loc[m][:], ps[:])
            for h in range(2):
                nc.sync.dma_start(
                    kv_in[h * c.CHR + m * c.KTW : h * c.CHR + (m + 1) * c.KTW, :],
                    kt_loc[m][:, h * 256 : (h + 1) * 256],
                )

        for rt in range(c.NT_R):
            for n in range(2):
                ps = projv_psum.tile([P, 512], F32, tag="projv", name="projv_ps")
                for k in range(c.NT_D):
                    nc.tensor.matmul(
                        ps[:],
                        xT[k][:, rt * P : (rt + 1) * P],
                        wv_sb[k][:, n * 512 : (n + 1) * 512],
                        start=(k == 0),
                        stop=(k == c.NT_D - 1),
                    )
                nc.vector.tensor_copy(vloc[rt][:, n * 512 : (n + 1) * 512], ps[:])
            h, lrt = divmod(rt, 2)
            base = h * c.CHR + c.NT_D * c.KTW
            nc.sync.dma_start(
                kv_in[base + lrt * c.VTW : base + (lrt + 1) * c.VTW, :], vloc[rt][:]
            )
            build_v_aug(rt, vloc[rt][:])

        # ---- collectives, back-to-back off kv_in ----
        # 1) intra-chip pairwise: buddy's whole block (fast link)
        nc.gpsimd.collective_compute(
            "AllGather",
            mybir.AluOpType.bypass,
            replica_groups=[[2 * g, 2 * g + 1] for g in range(N_CORES // 2)],
            ins=[kv_in[:].opt()],
            outs=[r1_g[:].opt()],
        )
        # 2+3) 4-way ring AllGather per key-half (for the far pair)
        for h in range(2):
            nc.gpsimd.collective_compute(
                "AllGather",
                mybir.AluOpType.bypass,
                replica_groups=[
                    list(range(GROUP)),
                    list(range(GROUP, 2 * GROUP)),
                ],
                ins=[kv_in[h * c.CHR : (h + 1) * c.CHR, :].opt()],
                outs=[cg[h][:].opt()],
            )

    rank = nc.sync.partition_id() % GROUP

    with (
        tc.tile_pool(name="vstage", bufs=2) as vstage,
        tc.tile_pool(name="pT", bufs=8) as pT_pool,
        tc.tile_pool(name="small", bufs=4) as small,
        tc.tile_pool(name="sc_psum", bufs=3, space="PSUM") as sc_psum,
        tc.tile_pool(name="att_psum", bufs=1, space="PSUM") as att_psum,
    ):
        def qproj(m):
            ps_full = sc_psum.tile([P, 2 * c.RPC], F32, tag="sc", name="q_ps")
            ps = ps_full[:, 0 : c.RPC]
            for k in range(c.NT_D):
                nc.tensor.matmul(
                    ps,
                    wq_sb[k][:, m * P : (m + 1) * P],
                    xT[k][:],
                    start=(k == 0),
                    stop=(k == c.NT_D - 1),
                )
            nc.vector.tensor_copy(qTe[m][0 : c.HD, :], ps[0 : c.HD, :])
            nc.vector.memset(qTe[m][c.HD : P, :], 0.0)
            nc.vector.memset(qTo[m][0 : c.HD, :], 0.0)
            nc.vector.tensor_copy(qTo[m][c.HD : P, :], ps[c.HD : P, :])

        def att_pass(kts, first, last, pre_pair=None):
            pend = []

            def drain():
                p, pts = pend.pop(0)
                he, ho = 2 * p, 2 * p + 1
                att = att_psum.tile([HD1, 2 * c.RPC], F32, tag="att", name="att")
                for idx, (pT, j) in enumerate(pts):
                    nc.tensor.matmul(
                        att[:, 0 : c.RPC],
                        v_aug(j)[:, he * HD1 : (he + 1) * HD1],
                        pT[:, 0 : c.RPC],
                        start=(idx == 0),
                        stop=(idx == len(pts) - 1),
                    )
                    nc.tensor.matmul(
                        att[:, c.RPC : 2 * c.RPC],
                        v_aug(j)[:, ho * HD1 : (ho + 1) * HD1],
                        pT[:, c.RPC : 2 * c.RPC],
                        start=(idx == 0),
                        stop=(idx == len(pts) - 1),
                    )
                if first:
                    nc.vector.tensor_copy(acc[p][:], att[:])
                else:
                    nc.vector.tensor_add(acc[p][:], att[:], acc[p][:])
                if last:
                    den = small.tile([1, 2 * c.RPC], F32, tag="den", name="den", bufs=1)
                    nc.vector.tensor_copy(den[:], acc[p][c.HD : HD1, :])
                    rcp = small.tile([1, 2 * c.RPC], F32, tag="rcp", name="rcp", bufs=1)
                    nc.vector.reciprocal_approx_fast(rcp[:], den[:])
                    rcpb = small.tile(
                        [c.HD, 2 * c.RPC], F32, tag="rcpb", name="rcpb", bufs=1
                    )
                    nc.gpsimd.partition_broadcast(rcpb[:], rcp[:])
                    nc.vector.tensor_mul(
                        attT[p][0 : c.HD, :],
                        acc[p][0 : c.HD, 0 : c.RPC],
                        rcpb[:, 0 : c.RPC],
                    )
                    nc.vector.tensor_mul(
                        attT[p][c.HD : P, :],
                        acc[p][0 : c.HD, c.RPC : 2 * c.RPC],
                        rcpb[:, c.RPC : 2 * c.RPC],
                    )

            for p in range(c.NT_D):
                if pre_pair is not None:
                    pre_pair(p)
                pts = []
                for kt_ap, j in kts:
                    sc = sc_psum.tile([P, 2 * c.RPC], F32, tag="sc", name="sc")
                    kt = kt_ap(p)
                    nc.tensor.matmul(
                        sc[:, 0 : c.RPC], kt, qTe[p][:], start=True, stop=True
                    )
                    nc.tensor.matmul(
                        sc[:, c.RPC : 2 * c.RPC], kt, qTo[p][:], start=True, stop=True
                    )
                    pT = pT_pool.tile([P, 2 * c.RPC], BF, tag="pT", name="pT")
                    nc.scalar.activation(
                        pT[:], sc[:], AF.Exp, scale=1.0 / float(np.sqrt(c.HD))
                    )
                    pts.append((pT, j))
                pend.append((p, pts))
                if len(pend) > 1:
                    drain()
            while pend:
                drain()

        def local_kts(js):
            return [
                (lambda p, _j=j: kt_loc[p][:, _j * P : (_j + 1) * P], j) for j in js
            ]

        # ---- own keys; Q^T projection rides along pair-by-pair ----
        att_pass(local_kts([0, 1]), first=True, last=False, pre_pair=qproj)
        att_pass(local_kts([2, 3]), first=False, last=False)

        # ---- unpacks (dynamic offsets pick the partner's block) ----
        def unpack_buddy():
            blk = (rank + 1) % 2
            for m in range(c.NT_D):
                for h in range(2):
                    nc.sync.dma_start(
                        kt_rem[m][:, h * 256 : (h + 1) * 256],
                        r1_g[:, :][bass.ts(blk * 32 + h * 16 + m, c.KTW), :],
                    )
            for rt in range(c.NT_R):
                h, lrt = divmod(rt, 2)
                vst = vstage.tile([P, c.D], BF, tag="vst", name="vst")
                nc.sync.dma_start(
                    vst[:],
                    r1_g[:, :][bass.ts(blk * 8 + h * 4 + 2 + lrt, c.VTW), :],
                )
                build_v_aug(4 + rt, vst[:])

        def unpack_far(h):
            fb = ((rank // 2) + 1) % 2 * 2
            for fs in range(2):
                blk = fb + fs
                s = 1 + fs
                for m in range(c.NT_D):
                    nc.sync.dma_start(
                        kt_rem[m][
                            :, s * c.RPC + h * 256 : s * c.RPC + (h + 1) * 256
                        ],
                        cg[h][:, :][bass.ts(blk * 16 + m, c.KTW), :],
                    )
                for lrt in range(2):
                    vst = vstage.tile([P, c.D], BF, tag="vst", name="vst")
                    nc.sync.dma_start(
                        vst[:],
                        cg[h][:, :][bass.ts(blk * 4 + 2 + lrt, c.VTW), :],
                    )
                    build_v_aug(4 + s * 4 + h * 2 + lrt, vst[:])

        def slot_kts(s, js):
            return [
                (
                    lambda p, _s=s, _j=j: kt_rem[p][
                        :, _s * c.RPC + _j * P : _s * c.RPC + (_j + 1) * P
                    ],
                    4 + s * 4 + j,
                )
                for j in js
            ]

        unpack_buddy()
        att_pass(slot_kts(0, [0, 1, 2, 3]), first=False, last=False)
        # far tiles reuse v_aug slots read by the local/buddy passes, so
        # their unpacks are emitted only after those passes.
        unpack_far(0)
        unpack_far(1)
        att_pass(slot_kts(1, [0, 1]) + slot_kts(2, [0, 1]), first=False, last=False)
        att_pass(slot_kts(1, [2, 3]) + slot_kts(2, [2, 3]), first=False, last=True)

        # ---- output projection + bias ----
        for rt in range(c.NT_R):
            out_sb = small.tile([P, c.D], F32, tag="outsb", name="outsb", bufs=1)
            for n in range(2):
                po = sc_psum.tile([P, 2 * c.RPC], F32, tag="sc", name="po")
                for k in range(c.NT_D):
                    nc.tensor.matmul(
                        po[:, 0:512],
                        attT[k][:, rt * P : (rt + 1) * P],
                        wo_sb[k][:, n * 512 : (n + 1) * 512],
                        start=(k == 0),
                        stop=False,
                    )
                nc.tensor.matmul(
                    po[:, 0:512],
                    ones_row[:],
                    bo_sb[:, n * 512 : (n + 1) * 512],
                    start=False,
                    stop=True,
                )
                nc.vector.tensor_copy(out_sb[:, n * 512 : (n + 1) * 512], po[:, 0:512])
            nc.sync.dma_start(out_ext[rt * P : (rt + 1) * P, :], out_sb[:])

    stack.close()


def build_nc(cfg):
    nc = bacc.Bacc(
        "TRN2", target_bir_lowering=False, debug=False, num_devices=N_CORES
    )
    c = cfg
    xT_in = nc.dram_tensor("xT", [c.D, c.RPC], BF, kind="ExternalInput")
    wq_in = nc.dram_tensor("Wq", [c.D, c.D], BF, kind="ExternalInput")
    wk_in = nc.dram_tensor("Wk", [c.D, c.D], BF, kind="ExternalInput")
    wv_in = nc.dram_tensor("Wv", [c.D, c.D], BF, kind="ExternalInput")
    wo_in = nc.dram_tensor("Wo", [c.D, c.D], BF, kind="ExternalInput")
    bo_in = nc.dram_tensor("bo", [1, c.D], BF, kind="ExternalInput")
    out_ext = nc.dram_tensor("out", [c.RPC, c.D], F32, kind="ExternalOutput")

    with tile.TileContext(nc) as tc:
        _body(
            tc, nc, cfg,
            xT_in.ap(), wq_in.ap(), wk_in.ap(), wv_in.ap(), wo_in.ap(),
            bo_in.ap(), out_ext.ap(),
        )
    nc.compile()
    return nc


_cached_nc = None


def _bf16(a):
    return np.ascontiguousarray(np.asarray(a, dtype=np.float32)).astype(
        ml_dtypes.bfloat16
    )


def prep_in_maps(c, x, Wq, Wk, Wv, Wo, bo):
    xf = np.ascontiguousarray(np.asarray(x, dtype=np.float32)).reshape(-1, c.D)
    wq, wk, wv, wo = _bf16(Wq), _bf16(Wk), _bf16(Wv), _bf16(Wo)
    bob = _bf16(bo).reshape(1, c.D)
    return [
        {
            "xT": np.ascontiguousarray(
                xf[cid * c.RPC : (cid + 1) * c.RPC].T.astype(ml_dtypes.bfloat16)
            ),
            "Wq": wq, "Wk": wk, "Wv": wv, "Wo": wo, "bo": bob,
        }
        for cid in range(N_CORES)
    ]


def kernel(x, Wq, Wk, Wv, Wo, bo):
    global _cached_nc
    c = FULL
    if _cached_nc is None:
        _cached_nc = build_nc(c)
    nc = _cached_nc

    in_maps = prep_in_maps(c, x, Wq, Wk, Wv, Wo, bo)
    res = run_bass_kernel_spmd(nc, in_maps, list(range(N_CORES)))
    out = np.concatenate([res.results[cid]["out"] for cid in range(N_CORES)], axis=0)
    return out.reshape(np.asarray(x).shape).astype(np.float32)
